# revision 4
# baseline (speedup 1.0000x reference)
"""Trainium2 Bass kernel for ClassicalGCN message passing, v2.

Reference computation:
    h   = tanh(x @ W1 + b1)                       # [N, HID]
    agg = segment_sum(edge_val * h[edge_col], edge_row, N)
    out = agg @ W2 + b2                           # [N, 1]

Algebraic rewrite: W2 commutes through the aggregation:
    s      = tanh(x @ W1 + b1) @ W2               # [N] per-node scalar
    out[i] = b2 + sum_{e: row[e]==i} val[e] * s[col[e]]

v2 design (vs the dma_gather baseline):
  Phase A (replicated on all 8 cores, bf16):
    - stream xT [128, 1024]-chunks, z = W1^T @ x on PE, tanh on ACT
    - the W2 contraction is done with two 128-wide "replicated" stationary
      matrices so each s value lands in ALL 128 PSUM partitions; ACT/DVE
      copies convert f32 -> bf16 into a replicated SBUF s-table
      [128 partitions, 50176] (viewed as 25088 uint32 bf16-pairs).
  Phase B (per core, its 6272 rows):
    - rows are degree-sorted globally and dealt into 49 rank-windows of
      1024 rows (8 cores x 128 partitions); window n uses ELL width
      W[n] = max degree in the window, so no overflow fixup is needed.
    - gpsimd ap_gather fetches the bf16 s-pair for each edge slot from
      SBUF (idx = col>>1, int16-safe); each gpsimd core serves its 16
      partitions with one shared index stream.
    - a per-slot bf16 mask pair ([val, 0] or [0, val]; zero for foreign
      partitions/padding) both selects the pair parity and applies
      edge_val; fused DVE tensor_tensor_reduce produces row sums per
      window. b2 is added on device.
  Host does index/mask prep (static given the graph) and inverse row
  permutation on the output; all FLOPs on x happen on device.
"""

import os

import numpy as np
import ml_dtypes

import concourse.bass as bass
import concourse.mybir as mybir
import concourse.tile as tile
from concourse import bacc
from concourse.bass_utils import run_bass_kernel_spmd
from concourse.tile_rust import add_dep_helper

N = 50000
E = 1600000
IN_DIM = 128
HID = 64
NCORES = 8

NPAD = 50176                 # nodes padded to 49*1024
NWIN = 49                    # degree-rank windows
RWIN = 1024                  # rows per window globally (8 cores x 128)
NT = NPAD // 2               # 25088 bf16-pair table entries
NI_MAX = 4096                # max idxs per ap_gather instruction

F32 = mybir.dt.float32
BF16 = mybir.dt.bfloat16
U32 = mybir.dt.uint32
I16 = mybir.dt.int16

BF = ml_dtypes.bfloat16

_LAST_RESULTS = {"exec_time_ns": None}
_PROGRAM_CACHE = {}


def _chunk_windows(W):
    """Group consecutive windows into ap_gather chunks of <= NI_MAX idxs."""
    chunks, cur, cur_ni = [], [], 0
    for n in range(NWIN):
        wni = 16 * W[n]
        assert wni <= NI_MAX, f"window {n} alone exceeds NI_MAX ({wni})"
        if cur and cur_ni + wni > NI_MAX:
            chunks.append(cur)
            cur, cur_ni = [], 0
        cur.append(n)
        cur_ni += wni
    chunks.append(cur)
    return chunks


def _build_program(W):
    TOT = 16 * sum(W)            # idx stream length per gpsimd core
    chunks = _chunk_windows(W)
    wmax = max(W)

    nc = bacc.Bacc("TRN2", target_bir_lowering=False, debug=False)

    xT = nc.dram_tensor("xT", [128, NPAD], BF16, kind="ExternalInput")
    W1d = nc.dram_tensor("W1d", [128, HID], BF16, kind="ExternalInput")
    b1c = nc.dram_tensor("b1c", [128, 1], F32, kind="ExternalInput")
    W2r = nc.dram_tensor("W2r", [128, 256], BF16, kind="ExternalInput")
    b2c = nc.dram_tensor("b2c", [128, 1], F32, kind="ExternalInput")
    idxs = nc.dram_tensor("idxs", [128, TOT // 16], I16, kind="ExternalInput")
    vm = nc.dram_tensor("vm", [128, 2 * TOT], BF16, kind="ExternalInput")
    outd = nc.dram_tensor("out", [128, NWIN], F32, kind="ExternalOutput")

    with tile.TileContext(nc) as tc:
        with tc.tile_pool(name="const", bufs=1) as cpool:
            W1_sb = cpool.tile([128, HID], BF16)
            nc.sync.dma_start(W1_sb[:], W1d[:, :])
            b1_sb = cpool.tile([128, 1], F32)
            nc.sync.dma_start(b1_sb[:], b1c[:, :])
            W2r_sb = cpool.tile([128, 256], BF16)
            nc.sync.dma_start(W2r_sb[:], W2r[:, :])
            b2_sb = cpool.tile([128, 1], F32)
            nc.sync.dma_start(b2_sb[:], b2c[:, :])
            idx_sb = cpool.tile([128, TOT // 16], I16)
            idx_dma = nc.sync.dma_start(idx_sb[:], idxs[:, :])
            tbl = cpool.tile([128, NT], U32)
            out_sb = cpool.tile([128, NWIN], F32)

            tbl_bf = tbl[:].bitcast(BF16)          # [128, 2*NT] bf16 view

            # ---- Phase A: replicated s-table build ----
            last_acopy = None
            last_vcopy = None
            with (
                tc.tile_pool(name="xload", bufs=4) as xpool,
                tc.tile_pool(name="thp", bufs=3) as thpool,
                tc.tile_pool(name="pz", bufs=3, space="PSUM") as pz,
                tc.tile_pool(name="ps", bufs=2, space="PSUM") as ps,
            ):
                # process window-chunks in pairs, grouping same-stationary
                # matmuls so PE reloads weights 3x (not 6x) per pair
                for i0 in range(0, NWIN, 2):
                    pair = [i for i in (i0, i0 + 1) if i < NWIN]
                    xts, zs, ths = {}, {}, {}
                    for i in pair:
                        xt = xpool.tile([128, 1024], BF16, tag="xt")
                        nc.sync.dma_start(xt[:],
                                          xT[:, 1024 * i : 1024 * (i + 1)])
                        xts[i] = xt
                    for i in pair:
                        z = pz.tile([128, 512], F32, tag="z")
                        nc.tensor.matmul(z[0:64, :], lhsT=W1_sb[:],
                                         rhs=xts[i][:, 0:512],
                                         start=True, stop=True)
                        nc.tensor.matmul(z[64:128, :], lhsT=W1_sb[:],
                                         rhs=xts[i][:, 512:1024],
                                         start=True, stop=True)
                        zs[i] = z
                    for i in pair:
                        th = thpool.tile([128, 512], BF16, tag="th")
                        nc.scalar.activation(th[:], zs[i][:],
                                             mybir.ActivationFunctionType.Tanh,
                                             bias=b1_sb[:, 0:1])
                        ths[i] = th
                    s1s = {}
                    for i in pair:
                        s1 = ps.tile([128, 512], F32, tag="s1")
                        nc.tensor.matmul(s1[:], lhsT=W2r_sb[:, 0:128],
                                         rhs=ths[i][:], start=True, stop=True)
                        s1s[i] = s1
                    s2s = {}
                    for i in pair:
                        s2 = ps.tile([128, 512], F32, tag="s2")
                        nc.tensor.matmul(s2[:], lhsT=W2r_sb[:, 128:256],
                                         rhs=ths[i][:], start=True, stop=True)
                        s2s[i] = s2
                    for i in pair:
                        last_acopy = nc.scalar.copy(
                            tbl_bf[:, 1024 * i : 1024 * i + 512], s1s[i][:])
                        last_vcopy = nc.vector.tensor_copy(
                            tbl_bf[:, 1024 * i + 512 : 1024 * (i + 1)],
                            s2s[i][:])

            # ---- Phase B: gather + masked window reduce ----
            tblv = tbl_bf.rearrange("p (n d) -> p n d", d=2)
            cum = np.concatenate([[0], np.cumsum([16 * w for w in W])])
            with (
                tc.tile_pool(name="gat", bufs=2) as gpool,
                tc.tile_pool(name="vml", bufs=2) as vpool,
            ):
                last_reader = [None, None]
                for ci, chunk in enumerate(chunks):
                    off = int(cum[chunk[0]])
                    ni = int(cum[chunk[-1] + 1]) - off
                    g = gpool.tile([128, NI_MAX, 2], BF16, tag="g")
                    ginst = nc.gpsimd.ap_gather(
                        out_ap=g[:, 0:ni, :],
                        in_ap=tblv,
                        idxs_ap=idx_sb[:, off // 16 : (off + ni) // 16],
                        channels=128,
                        num_elems=NT,
                        d=2,
                        num_idxs=ni,
                    )
                    add_dep_helper(ginst.ins, last_acopy.ins,
                                   reason="table complete (ACT copies)")
                    add_dep_helper(ginst.ins, last_vcopy.ins,
                                   reason="table complete (DVE copies)")
                    if last_reader[ci % 2] is not None:
                        add_dep_helper(ginst.ins, last_reader[ci % 2].ins,
                                       reason="g slot reuse WAR")
                    vmt = vpool.tile([128, 2 * NI_MAX], BF16, tag="vm")
                    nc.sync.dma_start(vmt[:, 0 : 2 * ni],
                                      vm[:, 2 * off : 2 * (off + ni)])
                    gflat = g[:].rearrange("p n d -> p (n d)")
                    minst = nc.vector.tensor_tensor(
                        out=gflat[:, 0 : 2 * ni],
                        in0=gflat[:, 0 : 2 * ni],
                        in1=vmt[:, 0 : 2 * ni],
                        op=mybir.AluOpType.mult,
                    )
                    add_dep_helper(minst.ins, ginst.ins,
                                   reason="gather data ready")
                    woff = 0
                    for n in chunk:
                        wlen = 16 * W[n] * 2
                        rinst = nc.vector.tensor_reduce(
                            out=out_sb[:, n : n + 1],
                            in_=gflat[:, woff : woff + wlen],
                            axis=mybir.AxisListType.X,
                            op=mybir.AluOpType.add,
                        )
                        woff += wlen
                        last_reader[ci % 2] = rinst

                nc.sync.dma_start(outd[:, :], out_sb[:])
    nc.compile()
    return nc, chunks


def _get_program(W):
    key = tuple(W)
    if key not in _PROGRAM_CACHE:
        _PROGRAM_CACHE[key] = _build_program(W)
    return _PROGRAM_CACHE[key]


def _preprocess(x, edge_row, edge_col, edge_val, W1, b1, W2, b2):
    deg = np.bincount(edge_row, minlength=NPAD).astype(np.int64)
    order = np.argsort(-deg, kind="stable")          # rank -> row
    rank_of = np.empty(NPAD, np.int64)
    rank_of[order] = np.arange(NPAD)

    W = deg[order[np.arange(NWIN) * RWIN]]           # max degree per window
    W = np.maximum(W, 1).astype(np.int64)
    W = (W + 1) // 2 * 2          # even widths: keeps every chunk's idx
    #                               count %32 and idx slice base 4B-aligned
    TOT = int(16 * W.sum())

    # CSR over rows
    eorder = np.argsort(edge_row, kind="stable")
    ers = edge_row[eorder].astype(np.int64)
    ecs = edge_col[eorder].astype(np.int64)
    evs = edge_val[eorder].astype(np.float32)
    starts = np.zeros(N + 1, np.int64)
    np.cumsum(deg[:N], out=starts[1:])
    w_in_row = np.arange(E, dtype=np.int64) - starts[ers]

    # per-edge placement
    rank = rank_of[ers]
    n_of = rank // RWIN
    jj = rank % RWIN
    core_of = jj // 128
    p_of = jj % 128
    g_of = p_of // 16
    pl_of = p_of % 16

    cum = np.zeros(NWIN + 1, np.int64)
    np.cumsum(16 * W, out=cum[1:])
    i_of = cum[n_of] + pl_of * W[n_of] + w_in_row    # stream position

    idx_row = 16 * g_of + (i_of % 16)
    idx_col = i_of // 16
    idx_val = (ecs >> 1).astype(np.int16)
    vm_pos = 2 * i_of + (ecs & 1)
    vm_val = evs

    idxs_cores, vm_cores = [], []
    for k in range(NCORES):
        m = core_of == k
        ik = np.zeros((128, TOT // 16), np.int16)
        ik[idx_row[m], idx_col[m]] = idx_val[m]
        vk = np.zeros((128, 2 * TOT), np.float32)
        vk[p_of[m], vm_pos[m]] = vm_val[m]
        idxs_cores.append(ik)
        vm_cores.append(vk.astype(BF))

    xT = np.zeros((128, NPAD), np.float32)
    xT[:, :N] = x.T
    xT = xT.astype(BF)

    W1h = W1.astype(BF)                              # [128, 64]
    b1c = np.tile(b1.astype(np.float32), 2).reshape(128, 1)
    W2r = np.zeros((128, 256), np.float32)
    W2r[0:64, 0:128] = W2[:, 0:1]                    # broadcast cols
    W2r[64:128, 128:256] = W2[:, 0:1]
    W2r = W2r.astype(BF)
    b2c = np.full((128, 1), np.float32(b2.reshape(-1)[0]), np.float32)

    return W, order, xT, W1h, b1c, W2r, b2c, idxs_cores, vm_cores


def kernel(x, edge_row, edge_col, edge_val, W1, b1, W2, b2):
    x = np.asarray(x, np.float32)
    edge_row = np.asarray(edge_row, np.int32)
    edge_col = np.asarray(edge_col, np.int32)
    edge_val = np.asarray(edge_val, np.float32)
    W1 = np.asarray(W1, np.float32)
    b1 = np.asarray(b1, np.float32)
    W2 = np.asarray(W2, np.float32)
    b2 = np.asarray(b2, np.float32)

    (W, order, xT, W1h, b1c, W2r, b2c, idxs_cores, vm_cores) = _preprocess(
        x, edge_row, edge_col, edge_val, W1, b1, W2, b2
    )
    nc, _ = _get_program(tuple(int(w) for w in W))

    in_maps = [
        {
            "xT": xT,
            "W1d": W1h,
            "b1c": b1c,
            "W2r": W2r,
            "b2c": b2c,
            "idxs": idxs_cores[k],
            "vm": vm_cores[k],
        }
        for k in range(NCORES)
    ]
    res = run_bass_kernel_spmd(
        nc,
        in_maps,
        core_ids=list(range(NCORES)),
        tmpdir=os.environ.get("GCN_TRACE_DIR") or None,
    )
    _LAST_RESULTS["exec_time_ns"] = res.exec_time_ns

    out = np.zeros((NPAD,), np.float32)
    ranks = np.arange(NPAD)
    rows = order[ranks]
    n_id = ranks // RWIN
    jj = ranks % RWIN
    core_id = jj // 128
    p_id = jj % 128
    dev = np.stack([np.asarray(res.results[k]["out"], np.float32)
                    for k in range(NCORES)])      # [core, 128, NWIN]
    out[rows] = dev[core_id, p_id, n_id]
    return (out[:N] + np.float32(b2.reshape(-1)[0])).reshape(N, 1)


# revision 5
# speedup vs baseline: 1.0180x; 1.0180x over previous
"""Trainium2 Bass kernel for ClassicalGCN message passing, v2.

Reference computation:
    h   = tanh(x @ W1 + b1)                       # [N, HID]
    agg = segment_sum(edge_val * h[edge_col], edge_row, N)
    out = agg @ W2 + b2                           # [N, 1]

Algebraic rewrite: W2 commutes through the aggregation:
    s      = tanh(x @ W1 + b1) @ W2               # [N] per-node scalar
    out[i] = b2 + sum_{e: row[e]==i} val[e] * s[col[e]]

v2 design (vs the dma_gather baseline):
  Phase A (replicated on all 8 cores, bf16):
    - stream xT [128, 1024]-chunks, z = W1^T @ x on PE, tanh on ACT
    - the W2 contraction is done with two 128-wide "replicated" stationary
      matrices so each s value lands in ALL 128 PSUM partitions; ACT/DVE
      copies convert f32 -> bf16 into a replicated SBUF s-table
      [128 partitions, 50176] (viewed as 25088 uint32 bf16-pairs).
  Phase B (per core, its 6272 rows):
    - rows are degree-sorted globally and dealt into 49 rank-windows of
      1024 rows (8 cores x 128 partitions); window n uses ELL width
      W[n] = max degree in the window, so no overflow fixup is needed.
    - gpsimd ap_gather fetches the bf16 s-pair for each edge slot from
      SBUF (idx = col>>1, int16-safe); each gpsimd core serves its 16
      partitions with one shared index stream.
    - a per-slot bf16 mask pair ([val, 0] or [0, val]; zero for foreign
      partitions/padding) both selects the pair parity and applies
      edge_val; fused DVE tensor_tensor_reduce produces row sums per
      window. b2 is added on device.
  Host does index/mask prep (static given the graph) and inverse row
  permutation on the output; all FLOPs on x happen on device.
"""

import os

import numpy as np
import ml_dtypes

import concourse.bass as bass
import concourse.mybir as mybir
import concourse.tile as tile
from concourse import bacc
from concourse.bass_utils import run_bass_kernel_spmd
from concourse.tile_rust import add_dep_helper

N = 50000
E = 1600000
IN_DIM = 128
HID = 64
NCORES = 8

NPAD = 50176                 # nodes padded to 49*1024
NWIN = 49                    # degree-rank windows
RWIN = 1024                  # rows per window globally (8 cores x 128)
NT = NPAD // 2               # 25088 bf16-pair table entries
NI_MAX = 6144                # max idxs per ap_gather instruction

F32 = mybir.dt.float32
BF16 = mybir.dt.bfloat16
U32 = mybir.dt.uint32
I16 = mybir.dt.int16

BF = ml_dtypes.bfloat16

_LAST_RESULTS = {"exec_time_ns": None}
_PROGRAM_CACHE = {}


def _chunk_windows(W):
    """Group consecutive windows into ap_gather chunks of <= NI_MAX idxs."""
    chunks, cur, cur_ni = [], [], 0
    for n in range(NWIN):
        wni = 16 * W[n]
        assert wni <= NI_MAX, f"window {n} alone exceeds NI_MAX ({wni})"
        if cur and cur_ni + wni > NI_MAX:
            chunks.append(cur)
            cur, cur_ni = [], 0
        cur.append(n)
        cur_ni += wni
    chunks.append(cur)
    return chunks


def _build_program(W):
    TOT = 16 * sum(W)            # idx stream length per gpsimd core
    chunks = _chunk_windows(W)
    wmax = max(W)

    nc = bacc.Bacc("TRN2", target_bir_lowering=False, debug=False)

    xT = nc.dram_tensor("xT", [128, NPAD], BF16, kind="ExternalInput")
    W1d = nc.dram_tensor("W1d", [128, HID], BF16, kind="ExternalInput")
    b1c = nc.dram_tensor("b1c", [128, 1], F32, kind="ExternalInput")
    W2r = nc.dram_tensor("W2r", [128, 256], BF16, kind="ExternalInput")
    b2c = nc.dram_tensor("b2c", [128, 1], F32, kind="ExternalInput")
    idxs = nc.dram_tensor("idxs", [128, TOT // 16], I16, kind="ExternalInput")
    vm = nc.dram_tensor("vm", [128, 2 * TOT], BF16, kind="ExternalInput")
    outd = nc.dram_tensor("out", [128, NWIN], F32, kind="ExternalOutput")

    with tile.TileContext(nc) as tc:
        with tc.tile_pool(name="const", bufs=1) as cpool:
            W1_sb = cpool.tile([128, HID], BF16)
            nc.sync.dma_start(W1_sb[:], W1d[:, :])
            b1_sb = cpool.tile([128, 1], F32)
            nc.sync.dma_start(b1_sb[:], b1c[:, :])
            W2r_sb = cpool.tile([128, 256], BF16)
            nc.sync.dma_start(W2r_sb[:], W2r[:, :])
            b2_sb = cpool.tile([128, 1], F32)
            nc.sync.dma_start(b2_sb[:], b2c[:, :])
            idx_sb = cpool.tile([128, TOT // 16], I16)
            idx_dma = nc.sync.dma_start(idx_sb[:], idxs[:, :])
            tbl = cpool.tile([128, NT], U32)
            out_sb = cpool.tile([128, NWIN], F32)

            tbl_bf = tbl[:].bitcast(BF16)          # [128, 2*NT] bf16 view

            # ---- Phase A: replicated s-table build ----
            last_acopy = None
            last_vcopy = None
            with (
                tc.tile_pool(name="xload", bufs=4) as xpool,
                tc.tile_pool(name="thp", bufs=3) as thpool,
                tc.tile_pool(name="pz", bufs=3, space="PSUM") as pz,
                tc.tile_pool(name="ps", bufs=2, space="PSUM") as ps,
            ):
                # process window-chunks in pairs, grouping same-stationary
                # matmuls so PE reloads weights 3x (not 6x) per pair
                for i0 in range(0, NWIN, 2):
                    pair = [i for i in (i0, i0 + 1) if i < NWIN]
                    xts, zs, ths = {}, {}, {}
                    for i in pair:
                        xt = xpool.tile([128, 1024], BF16, tag="xt")
                        nc.sync.dma_start(xt[:],
                                          xT[:, 1024 * i : 1024 * (i + 1)])
                        xts[i] = xt
                    for i in pair:
                        z = pz.tile([128, 512], F32, tag="z")
                        nc.tensor.matmul(z[0:64, :], lhsT=W1_sb[:],
                                         rhs=xts[i][:, 0:512],
                                         start=True, stop=True)
                        nc.tensor.matmul(z[64:128, :], lhsT=W1_sb[:],
                                         rhs=xts[i][:, 512:1024],
                                         start=True, stop=True)
                        zs[i] = z
                    for i in pair:
                        th = thpool.tile([128, 512], BF16, tag="th")
                        nc.scalar.activation(th[:], zs[i][:],
                                             mybir.ActivationFunctionType.Tanh,
                                             bias=b1_sb[:, 0:1])
                        ths[i] = th
                    s1s = {}
                    for i in pair:
                        s1 = ps.tile([128, 512], F32, tag="s1")
                        nc.tensor.matmul(s1[:], lhsT=W2r_sb[:, 0:128],
                                         rhs=ths[i][:], start=True, stop=True)
                        s1s[i] = s1
                    s2s = {}
                    for i in pair:
                        s2 = ps.tile([128, 512], F32, tag="s2")
                        nc.tensor.matmul(s2[:], lhsT=W2r_sb[:, 128:256],
                                         rhs=ths[i][:], start=True, stop=True)
                        s2s[i] = s2
                    for i in pair:
                        last_acopy = nc.scalar.copy(
                            tbl_bf[:, 1024 * i : 1024 * i + 512], s1s[i][:])
                        last_vcopy = nc.vector.tensor_copy(
                            tbl_bf[:, 1024 * i + 512 : 1024 * (i + 1)],
                            s2s[i][:])

            # ---- Phase B: gather + masked window reduce ----
            tblv = tbl_bf.rearrange("p (n d) -> p n d", d=2)
            cum = np.concatenate([[0], np.cumsum([16 * w for w in W])])
            with (
                tc.tile_pool(name="gat", bufs=2) as gpool,
                tc.tile_pool(name="vml", bufs=2) as vpool,
            ):
                last_reader = [None, None]
                for ci, chunk in enumerate(chunks):
                    off = int(cum[chunk[0]])
                    ni = int(cum[chunk[-1] + 1]) - off
                    g = gpool.tile([128, NI_MAX, 2], BF16, tag="g")
                    ginst = nc.gpsimd.ap_gather(
                        out_ap=g[:, 0:ni, :],
                        in_ap=tblv,
                        idxs_ap=idx_sb[:, off // 16 : (off + ni) // 16],
                        channels=128,
                        num_elems=NT,
                        d=2,
                        num_idxs=ni,
                    )
                    add_dep_helper(ginst.ins, last_acopy.ins,
                                   reason="table complete (ACT copies)")
                    add_dep_helper(ginst.ins, last_vcopy.ins,
                                   reason="table complete (DVE copies)")
                    if last_reader[ci % 2] is not None:
                        add_dep_helper(ginst.ins, last_reader[ci % 2].ins,
                                       reason="g slot reuse WAR")
                    vmt = vpool.tile([128, 2 * NI_MAX], BF16, tag="vm")
                    nc.sync.dma_start(vmt[:, 0 : 2 * ni],
                                      vm[:, 2 * off : 2 * (off + ni)])
                    gflat = g[:].rearrange("p n d -> p (n d)")
                    minst = nc.vector.tensor_tensor(
                        out=gflat[:, 0 : 2 * ni],
                        in0=gflat[:, 0 : 2 * ni],
                        in1=vmt[:, 0 : 2 * ni],
                        op=mybir.AluOpType.mult,
                    )
                    add_dep_helper(minst.ins, ginst.ins,
                                   reason="gather data ready")
                    woff = 0
                    for n in chunk:
                        wlen = 16 * W[n] * 2
                        rinst = nc.vector.tensor_reduce(
                            out=out_sb[:, n : n + 1],
                            in_=gflat[:, woff : woff + wlen],
                            axis=mybir.AxisListType.X,
                            op=mybir.AluOpType.add,
                        )
                        woff += wlen
                        last_reader[ci % 2] = rinst

                nc.sync.dma_start(outd[:, :], out_sb[:])
    nc.compile()
    return nc, chunks


def _get_program(W):
    key = tuple(W)
    if key not in _PROGRAM_CACHE:
        _PROGRAM_CACHE[key] = _build_program(W)
    return _PROGRAM_CACHE[key]


def _preprocess(x, edge_row, edge_col, edge_val, W1, b1, W2, b2):
    deg = np.bincount(edge_row, minlength=NPAD).astype(np.int64)
    order = np.argsort(-deg, kind="stable")          # rank -> row
    rank_of = np.empty(NPAD, np.int64)
    rank_of[order] = np.arange(NPAD)

    W = deg[order[np.arange(NWIN) * RWIN]]           # max degree per window
    W = np.maximum(W, 1).astype(np.int64)
    W = (W + 1) // 2 * 2          # even widths: keeps every chunk's idx
    #                               count %32 and idx slice base 4B-aligned
    TOT = int(16 * W.sum())

    # CSR over rows
    eorder = np.argsort(edge_row, kind="stable")
    ers = edge_row[eorder].astype(np.int64)
    ecs = edge_col[eorder].astype(np.int64)
    evs = edge_val[eorder].astype(np.float32)
    starts = np.zeros(N + 1, np.int64)
    np.cumsum(deg[:N], out=starts[1:])
    w_in_row = np.arange(E, dtype=np.int64) - starts[ers]

    # per-edge placement
    rank = rank_of[ers]
    n_of = rank // RWIN
    jj = rank % RWIN
    core_of = jj // 128
    p_of = jj % 128
    g_of = p_of // 16
    pl_of = p_of % 16

    cum = np.zeros(NWIN + 1, np.int64)
    np.cumsum(16 * W, out=cum[1:])
    i_of = cum[n_of] + pl_of * W[n_of] + w_in_row    # stream position

    idx_row = 16 * g_of + (i_of % 16)
    idx_col = i_of // 16
    idx_val = (ecs >> 1).astype(np.int16)
    vm_pos = 2 * i_of + (ecs & 1)
    vm_val = evs

    idxs_cores, vm_cores = [], []
    for k in range(NCORES):
        m = core_of == k
        ik = np.zeros((128, TOT // 16), np.int16)
        ik[idx_row[m], idx_col[m]] = idx_val[m]
        vk = np.zeros((128, 2 * TOT), np.float32)
        vk[p_of[m], vm_pos[m]] = vm_val[m]
        idxs_cores.append(ik)
        vm_cores.append(vk.astype(BF))

    xT = np.zeros((128, NPAD), np.float32)
    xT[:, :N] = x.T
    xT = xT.astype(BF)

    W1h = W1.astype(BF)                              # [128, 64]
    b1c = np.tile(b1.astype(np.float32), 2).reshape(128, 1)
    W2r = np.zeros((128, 256), np.float32)
    W2r[0:64, 0:128] = W2[:, 0:1]                    # broadcast cols
    W2r[64:128, 128:256] = W2[:, 0:1]
    W2r = W2r.astype(BF)
    b2c = np.full((128, 1), np.float32(b2.reshape(-1)[0]), np.float32)

    return W, order, xT, W1h, b1c, W2r, b2c, idxs_cores, vm_cores


def kernel(x, edge_row, edge_col, edge_val, W1, b1, W2, b2):
    x = np.asarray(x, np.float32)
    edge_row = np.asarray(edge_row, np.int32)
    edge_col = np.asarray(edge_col, np.int32)
    edge_val = np.asarray(edge_val, np.float32)
    W1 = np.asarray(W1, np.float32)
    b1 = np.asarray(b1, np.float32)
    W2 = np.asarray(W2, np.float32)
    b2 = np.asarray(b2, np.float32)

    (W, order, xT, W1h, b1c, W2r, b2c, idxs_cores, vm_cores) = _preprocess(
        x, edge_row, edge_col, edge_val, W1, b1, W2, b2
    )
    nc, _ = _get_program(tuple(int(w) for w in W))

    in_maps = [
        {
            "xT": xT,
            "W1d": W1h,
            "b1c": b1c,
            "W2r": W2r,
            "b2c": b2c,
            "idxs": idxs_cores[k],
            "vm": vm_cores[k],
        }
        for k in range(NCORES)
    ]
    res = run_bass_kernel_spmd(
        nc,
        in_maps,
        core_ids=list(range(NCORES)),
        tmpdir=os.environ.get("GCN_TRACE_DIR") or None,
    )
    _LAST_RESULTS["exec_time_ns"] = res.exec_time_ns

    out = np.zeros((NPAD,), np.float32)
    ranks = np.arange(NPAD)
    rows = order[ranks]
    n_id = ranks // RWIN
    jj = ranks % RWIN
    core_id = jj // 128
    p_id = jj % 128
    dev = np.stack([np.asarray(res.results[k]["out"], np.float32)
                    for k in range(NCORES)])      # [core, 128, NWIN]
    out[rows] = dev[core_id, p_id, n_id]
    return (out[:N] + np.float32(b2.reshape(-1)[0])).reshape(N, 1)
